# revision 40
# baseline (speedup 1.0000x reference)
"""Cross-attention Trainium2 Bass kernel (fp8 DoubleRow attention).

Problem (per batch element, fp32):
    q = x1 @ Wq + bq; k = x2 @ Wk + bk; v = x2 @ Wv + bv
    out = softmax(q k^T / sqrt(512)) @ v        with LQ = LK = 2048, D = 512

Sharding: batch (B=8) across the 8 NeuronCores, one batch element per core;
weights replicated. Full inputs in, full output out.

Per-core plan:
  - Projections in f32r (full precision path), but weights/biases are
    pre-scaled by S=64 host-side so the q/k/v results can be stored as
    fp8e4 (e4m3) without denormal loss: q'=64q, k'=64k, v'=64v.
  - X1^T / X2^T via PE-mode transposes.
  - scores'^T[k,q] = K'^T.T @ Q'^T in fp8e4 with perf_mode=DoubleRow
    (2 contraction chunks per instruction, 2 rows/cycle): scores' =
    4096*scores, so exp scale = (1/sqrt(D))/4096.
  - P^T handled as a residual: ACT writes p = exp(..) to an f32 staging
    tile, then DVE/GpSimd write r = p - 1 as fp8e4 (r ~ +-0.2 quantizes
    ~5x better than p ~ 1.0). All fp8 stores use tensor_tensor ops --
    tensor_scalar with fp8 out is ~4.7x slower on DVE.
  - denominator: ones(64)-lhsT DoubleRow matmuls over r (emitted after
    the scores loop so they never head-of-line block the PE queue) give
    64*sum(r); +64*2048 is added as an immediate during the PSUM->SBUF
    copy, giving den = 64*sum(p). DRAM-bounce to per-partition columns.
  - out numerator: PSUM group per out tile = bf16 rank-1 matmul seeding
    colsum(V') (handles the "+1" of p = 1 + r) followed by fp8
    DoubleRow matmuls accumulating R^T.T @ V', split into half
    contractions (A: k-tiles 0-7, B: 8-15) so the A halves overlap the
    exp/subtract tail. Normalize by 1/(64 sum p) on DVE; fp32 out.
"""
import sys

sys.path.insert(0, "/opt/trn_rl_repo")
import numpy as np
import concourse.bass as bass
import concourse.tile as tile
from concourse.tile import add_dep_helper
import concourse.bacc as bacc
from concourse import mybir
from concourse.bass_utils import run_bass_kernel_spmd
from concourse.masks import make_identity

B, LQ, LK, D = 8, 2048, 2048, 512
P = 128
NKT = LK // P          # 16 k-tiles
NDC = D // P           # 4 d-chunks
NQB = LQ // 512        # 4 q-blocks of 512
NCORES = 8
WSCALE = 64.0          # host-side weight/bias scale (fp8 denormal avoidance)
SCALE = float(1.0 / np.sqrt(np.float32(D)) / (WSCALE * WSCALE))
DEN_CONST = float(WSCALE * LK)   # 64 * sum_k 1 (the "+1" part of p = 1+r)

f32 = mybir.dt.float32
f32r = mybir.dt.float32r
bf16 = mybir.dt.bfloat16
f8 = mybir.dt.float8e4
ts = bass.ts
Exp = mybir.ActivationFunctionType.Exp
DR = mybir.MatmulPerfMode.DoubleRow

_CACHE = {}


def _round_to(nc, pool, src_ap, shape, tag):
    """Stage-copy an f32 AP into a fresh f32r tile (DVE rounds on writeback)."""
    t = pool.tile(shape, f32r, tag=tag, name=f"r_{tag}")
    nc.vector.tensor_copy(t[:], src_ap)
    return t


def _build():
    nc = bacc.Bacc("TRN2", target_bir_lowering=False, debug=False,
                   num_devices=NCORES)
    X1 = nc.declare_dram_parameter("x1", [LQ, D], f32, isOutput=False)
    X2 = nc.declare_dram_parameter("x2", [LK, D], f32, isOutput=False)
    WQ = nc.declare_dram_parameter("wq", [D, D], f32, isOutput=False)
    WK = nc.declare_dram_parameter("wk", [D, D], f32, isOutput=False)
    WV = nc.declare_dram_parameter("wv", [D, D], f32, isOutput=False)
    BV = nc.declare_dram_parameter("bv", [D], f32, isOutput=False)
    BR = nc.declare_dram_parameter("brows", [1, 8 * P], f32, isOutput=False)
    OUT = nc.declare_dram_parameter("out", [LQ, D], f32, isOutput=True)
    DEN = nc.dram_tensor("den_scratch", [NQB, 512], f32)

    with tile.TileContext(nc) as tc:
        with (
            tc.tile_pool(name="const", bufs=1) as cpool,
            tc.tile_pool(name="wts", bufs=1) as wpool,
            tc.tile_pool(name="stage", bufs=2) as stage,
            tc.tile_pool(name="wstage", bufs=1) as wstage,
            tc.tile_pool(name="xtp", bufs=2) as xtp,
            tc.tile_pool(name="bfx", bufs=3) as bfx,
            tc.tile_pool(name="qtp", bufs=2) as qtp,
            tc.tile_pool(name="big", bufs=1) as big,
            tc.tile_pool(name="rtp", bufs=2) as rtp,
            tc.tile_pool(name="ptmp", bufs=4) as ppool,
            tc.tile_pool(name="obuf", bufs=2) as obuf,
            tc.tile_pool(name="psA", bufs=2, space="PSUM") as psA,
            tc.tile_pool(name="psB", bufs=2, space="PSUM") as psB,
            tc.tile_pool(name="psO", bufs=3, space="PSUM") as psO,
            tc.tile_pool(name="psD", bufs=1, space="PSUM") as psD,
        ):
            # ---- startup critical path: x2 block 0, Wk, identity first ----
            identf = cpool.tile([P, P], f32, tag="identf")
            make_identity(nc, identf[:])
            ident = cpool.tile([P, P], bf16, tag="ident_bf")
            nc.vector.tensor_copy(ident[:], identf[:])

            dma_insts = {}

            def load_x_block(X, blk, qsel, after=None):
                """Four 256KB DMAs: rows blk*512..+512 as [128, 4, 512].
                Per-row-tile DMAs let the first transpose start ~4x sooner
                than a single 1MB transfer."""
                xin = stage.tile([P, 4, D], f32, tag="xin",
                                 name=f"xin_{qsel}_{blk}")
                src = X.ap().rearrange("(b t p) d -> b t p d", p=P, t=4)[blk]
                eng = nc.sync if (blk + qsel) % 2 == 0 else nc.scalar
                di = None
                for t in range(4):
                    di = eng.dma_start(xin[:, t, :], src[t])
                    if after is not None:
                        add_dep_helper(di.ins, dma_insts[after].ins,
                                       reason="stagger DMA bandwidth")
                dma_insts[f"x{qsel}_{blk}"] = di
                return xin

            def load_w(W, name, qsel, after=None):
                """One 1MB DMA + one [128, 2048] rounding cast."""
                wst = wstage.tile([P, 4, D], f32, tag="wst",
                                  name=f"wst_{name}")
                src = W.ap().rearrange("(c p) n -> p c n", p=P)
                eng = nc.sync if qsel % 2 == 0 else nc.scalar
                di = eng.dma_start(wst[:], src)
                if after is not None:
                    add_dep_helper(di.ins, dma_insts[after].ins,
                                   reason="stagger DMA bandwidth")
                dma_insts[name] = di
                t = wpool.tile([P, 4, D], bf16, tag=name, name=f"r_{name}")
                nc.vector.tensor_copy(t[:], wst[:])
                return t

            # wave 1: x2 block 0 + Wk get the full pipe; later waves chain
            xin2_0 = load_x_block(X2, 0, 0)
            wk_r = load_w(WK, "wk", 1)
            wv_r = load_w(WV, "wv", 0)
            wq_r = None  # loaded after the X2 stream (only prep_q needs it)

            # persistent K'^T and V' (fp8, chunk/tile-paired for DoubleRow)
            ktf = big.tile([P, NDC, LK], f8, tag="ktf", name="ktf")
            vt = big.tile([P, NKT, D], f8, tag="vt", name="vt")

            # ---- small constants ----
            # DoubleRow weights APs need pair-dim stride % 16 bytes == 0.
            # M=16 (vs 2): avoids a tiny-stationary pathology; rows identical.
            ones_den_t = cpool.tile([P, 2, 16], f8, tag="ones_den")
            nc.vector.memset(ones_den_t[:], WSCALE)
            ones_den = ones_den_t[:, :, :]
            ones_cs_t = cpool.tile([P, 2, 16], f8, tag="ones_cs")
            nc.vector.memset(ones_cs_t[:], 1.0)
            ones_cs = ones_cs_t[:, :, 0:1]

            bv_f = cpool.tile([1, D], f32, tag="bv_f")
            nc.scalar.dma_start(bv_f[:], BV[:].unsqueeze(0))
            onesr_f = cpool.tile([1, P], f32, tag="onesr_f")
            nc.vector.memset(onesr_f[:], 1.0)
            ones_row = _round_to(nc, cpool, onesr_f[:], [1, P], "ones_row")
            bv_row = _round_to(nc, cpool, bv_f[:], [1, D], "bv_row")
            brows_f = cpool.tile([1, 8 * P], f32, tag="brows_f")
            nc.scalar.dma_start(brows_f[:], BR[:])
            brows_r = _round_to(nc, cpool, brows_f[:], [1, 8 * P], "brows_r")
            ones512f = cpool.tile([1, 512], f32, tag="ones512f")
            nc.vector.memset(ones512f[:], 1.0)
            ones512 = _round_to(nc, cpool, ones512f[:], [1, 512], "ones512")
            dconst = cpool.tile([1, 1], f32, tag="dconst")
            nc.vector.memset(dconst[:], DEN_CONST)
            bqb, bkb = [], []
            bv_bcast = cpool.tile([P, D], f32, tag="bv_bcast")
            onesb = cpool.tile([P, 512], f32, tag="onesb")
            nc.vector.memset(onesb[:], 1.0)

            def build_bias_bcasts():
                """PE rank-1 broadcasts for bv / bq / bk -> [128, 512] f32
                tiles. Emitted AFTER the first transposes so the small bias
                DMAs/casts never head-of-line-block the PE queue."""
                bvb_ps = psA.tile([P, D], f32, tag="tp", name="bvb_ps")
                nc.tensor.matmul(bvb_ps[:], ones_row[:], bv_row[:],
                                 start=True, stop=True)
                nc.vector.tensor_copy(bv_bcast[:], bvb_ps[:])
                for i in range(8):
                    bps = psA.tile([P, D], f32, tag="tp", name=f"bps_{i}")
                    nc.tensor.matmul(bps[:], brows_r[:, ts(i, P)],
                                     ones512[:], start=True, stop=True)
                    bt = cpool.tile([P, 512], f32, tag=f"bb{i}",
                                    name=f"bb_{i}")
                    nc.vector.tensor_copy(bt[:], bps[:])
                    (bqb if i < 4 else bkb).append(bt)

            def transpose_tp(xin, tp, chunks):
                """Cast xin[:, tp, :] to bf16 (ACT), then PE-transpose the
                4 [128,128] sub-tiles into per-d-chunk bf16 columns at
                column tp*128 of the [P, NDC, 512] chunk tile. bf16
                transposes stream 1 cyc/row (f32: 2) and the 2-byte
                PSUM->SBUF copies hit the DVE 2x path."""
                xbf = bfx.tile([P, 512], bf16, tag="xbf")
                nc.scalar.copy(xbf[:], xin[:, tp, :])
                for ci in range(NDC):
                    tps = psA.tile([P, P], bf16, tag="tp")
                    nc.tensor.transpose(tps[:], xbf[:, ts(ci, P)],
                                        ident[:])
                    nc.vector.tensor_copy(chunks[:, ci, ts(tp, P)],
                                          tps[:])

            # ---------------- phase A1: X2 -> K'^T, V' ----------------
            def emit_v(x2t, kb, tp):
                t = kb * 4 + tp
                mm = psB.tile([P, 512], f32, tag="mm")
                for cj in range(NDC):
                    nc.tensor.matmul(mm[:], x2t[:, cj, ts(tp, P)],
                                     wv_r[:, cj, :], start=(cj == 0),
                                     stop=(cj == NDC - 1))
                nc.vector.tensor_add(vt[:, t, :], mm[:], bv_bcast[:])

            def emit_k(x2t, kb, ci):
                mm = psB.tile([P, 512], f32, tag="mm")
                for cj in range(NDC):
                    nc.tensor.matmul(mm[:], wk_r[:, cj, ts(ci, P)],
                                     x2t[:, cj, :], start=(cj == 0),
                                     stop=(cj == NDC - 1))
                nc.vector.tensor_add(ktf[:, ci, ts(kb, 512)],
                                     mm[:], bkb[ci][:])

            for kb in range(4):
                xin = xin2_0 if kb == 0 else load_x_block(X2, kb, 0)
                x2t = xtp.tile([P, NDC, 512], bf16, tag="x2t",
                               name=f"x2t_{kb}")
                if kb == 0:
                    # wk arrives before wv: transposes, then K^T, then V
                    for tp in range(4):
                        transpose_tp(xin, tp, x2t)
                    build_bias_bcasts()
                    for ci in range(NDC):
                        emit_k(x2t, kb, ci)
                    for tp in range(4):
                        emit_v(x2t, kb, tp)
                else:
                    for tp in range(4):
                        transpose_tp(xin, tp, x2t)
                        emit_v(x2t, kb, tp)
                    for ci in range(NDC):
                        emit_k(x2t, kb, ci)

            wq_r = load_w(WQ, "wq", 0)

            # colsum(V') row: seeds every out-tile PSUM group (p = 1 + r).
            # bf16 rank-1 (vs f32r) halves the seed matmul cost; csum ~1e3
            # so bf16's 0.4% rounding is ~4e-5 relative on out.
            cs_ps = psB.tile([P, 512], f32, tag="mm", name="cs_ps")
            for j in range(NKT // 2):
                nc.tensor.matmul(cs_ps[0:1, :], ones_cs,
                                 vt[:, 2 * j:2 * j + 2, :],
                                 start=(j == 0), stop=(j == NKT // 2 - 1),
                                 perf_mode=DR)
            csum_row = cpool.tile([1, 512], bf16, tag="csum_row")
            nc.vector.tensor_copy(csum_row[:], cs_ps[0:1, :])
            ones_row_bf = cpool.tile([1, P], bf16, tag="ones_row_bf")
            nc.vector.memset(ones_row_bf[:], 1.0)

            # ---------- phase A2+B per q-block: Q^T, scores, softmax, out ----
            def prep_q(qb):
                """x1 load + transposes + Q'^T (fp8) for block qb."""
                xin = load_x_block(X1, qb, 1)
                x1t = xtp.tile([P, NDC, 512], bf16, tag="x1t",
                               name=f"x1t_{qb}")
                for tp in range(4):
                    transpose_tp(xin, tp, x1t)
                qt = qtp.tile([P, NDC, 512], f8, tag="qt", name=f"qt_{qb}")
                for ci in range(NDC):
                    mm = psB.tile([P, 512], f32, tag="mm")
                    for cj in range(NDC):
                        nc.tensor.matmul(mm[:], wq_r[:, cj, ts(ci, P)],
                                         x1t[:, cj, :], start=(cj == 0),
                                         stop=(cj == NDC - 1))
                    nc.vector.tensor_add(qt[:, ci, :], mm[:], bqb[ci][:])
                return qt

            qt_next = prep_q(0)
            for qb in range(NQB):
                qt = qt_next

                # scores'^T -> exp -> r = p-1 (fp8)
                rts = rtp.tile([P, NKT, 512], f8, tag="rts",
                               name=f"rts_{qb}")
                dps = psD.tile([16, 512], f32, tag="d")
                last = qb == NQB - 1
                for t in range(NKT):
                    smm = psB.tile([P, 512], f32, tag="mm")
                    for j in range(NDC // 2):
                        nc.tensor.matmul(smm[:],
                                         ktf[:, 2 * j:2 * j + 2, ts(t, P)],
                                         qt[:, 2 * j:2 * j + 2, :],
                                         start=(j == 0),
                                         stop=(j == NDC // 2 - 1),
                                         perf_mode=DR)
                    pt = ppool.tile([P, 512], f32, tag="pt")
                    nc.scalar.activation(pt[:], smm[:], Exp, scale=SCALE)
                    # gpsimd runs ~2x slower than DVE on the fp8 subtract;
                    # giving it every 3rd tile balances the two queues
                    eng = nc.gpsimd if t % 3 == 2 else nc.vector
                    eng.tensor_sub(rts[:, t, :], pt[:], onesb[:])
                    if last and t % 2 == 1:
                        nc.tensor.matmul(dps[:], ones_den,
                                         rts[:, t - 1:t + 1, :],
                                         start=(t == 1),
                                         stop=(t == NKT - 1),
                                         perf_mode=DR)


                # PV split into half-contractions: the A halves only need
                # rts tiles 0-7, so they overlap the exp/subtract tail of
                # tiles 8-15 instead of the PE idling on the full chain.
                def pv_a(s):
                    ops = psO.tile([P, 512], f32, tag="o",
                                   name=f"ops_{qb}_{s}")
                    nc.tensor.matmul(ops[:], ones_row_bf[:], csum_row[:],
                                     start=True, stop=False)
                    for j in range(NKT // 4):
                        nc.tensor.matmul(ops[:],
                                         rts[:, 2 * j:2 * j + 2, ts(s, P)],
                                         vt[:, 2 * j:2 * j + 2, :],
                                         start=False, stop=False,
                                         perf_mode=DR)
                    return ops

                def pv_b(s, ops):
                    for j in range(NKT // 4, NKT // 2):
                        nc.tensor.matmul(ops[:],
                                         rts[:, 2 * j:2 * j + 2, ts(s, P)],
                                         vt[:, 2 * j:2 * j + 2, :],
                                         start=False,
                                         stop=(j == NKT // 2 - 1),
                                         perf_mode=DR)
                    osb = obuf.tile([P, 512], f32, tag="osb")
                    nc.vector.tensor_scalar_mul(osb[:], ops[:],
                                                rec[:, s:s + 1])
                    nc.sync.dma_start(OUT[ts(qb * 4 + s, P), :], osb[:])

                ops0, ops1 = pv_a(0), pv_a(1)

                # den matmuls HERE: after pv_a(0/1) so the PE has useful
                # work while the exp/subtract tail (rts 8-15) completes,
                # and before prep_q so the small den DMAs beat the next
                # block's 1MB x1 load into the scalar queue.
                if not last:
                    for th in range(NKT // 2):
                        nc.tensor.matmul(dps[:], ones_den,
                                         rts[:, 2 * th:2 * th + 2, :],
                                         start=(th == 0),
                                         stop=(th == NKT // 2 - 1),
                                         perf_mode=DR)
                den_sb = cpool.tile([1, 512], f32, tag="den_sb",
                                    name=f"den_sb_{qb}")
                nc.vector.tensor_scalar_add(den_sb[:], dps[0:1, :], DEN_CONST)
                nc.scalar.dma_start(DEN[qb].unsqueeze(0), den_sb[:])
                den_cols = obuf.tile([P, 4], f32, tag="den_cols")
                for s in range(4):
                    nc.scalar.dma_start(den_cols[:, s:s + 1],
                                        DEN[qb, ts(s, P)].unsqueeze(1))
                rec = obuf.tile([P, 4], f32, tag="rec")
                nc.vector.reciprocal(rec[:], den_cols[:])

                if qb + 1 < NQB:
                    qt_next = prep_q(qb + 1)

                pv_b(0, ops0)
                pv_b(1, ops1)
                ops2, ops3 = pv_a(2), pv_a(3)
                pv_b(2, ops2)
                pv_b(3, ops3)

    nc.compile()
    return nc


def _get_nc():
    if "nc" not in _CACHE:
        _CACHE["nc"] = _build()
    return _CACHE["nc"]


def kernel(x_1, x_2, Wq, bq, Wk, bk, Wv, bv, **_run_kwargs):
    x_1 = np.ascontiguousarray(np.asarray(x_1, dtype=np.float32))
    x_2 = np.ascontiguousarray(np.asarray(x_2, dtype=np.float32))
    S = np.float32(WSCALE)
    Wq = np.ascontiguousarray(np.asarray(Wq, dtype=np.float32) * S)
    bq = np.ascontiguousarray(np.asarray(bq, dtype=np.float32) * S)
    Wk = np.ascontiguousarray(np.asarray(Wk, dtype=np.float32) * S)
    bk = np.ascontiguousarray(np.asarray(bk, dtype=np.float32) * S)
    Wv = np.ascontiguousarray(np.asarray(Wv, dtype=np.float32) * S)
    bv = np.ascontiguousarray(np.asarray(bv, dtype=np.float32) * S)

    brows = np.concatenate([bq, bk]).reshape(1, 8 * P)
    brows = np.ascontiguousarray(brows.astype(np.float32))

    nc = _get_nc()
    in_maps = [
        {"x1": x_1[c], "x2": x_2[c], "wq": Wq,
         "wk": Wk, "wv": Wv, "bv": bv, "brows": brows}
        for c in range(NCORES)
    ]
    res = run_bass_kernel_spmd(nc, in_maps, list(range(NCORES)),
                               **_run_kwargs)
    if _run_kwargs:
        _CACHE["last_results"] = res
    return np.stack([res.results[c]["out"] for c in range(NCORES)])
